# revision 7
# baseline (speedup 1.0000x reference)
"""BLOBLoss Trainium2 kernel.

Math background (mirrors the reference):
  scores[r,c] = mean_k(refine[k,r,c+1]) thresholded at 0.3, masked to valid classes.
  M[y,x,c]   = sum_r scores[r,c] * [y1_r<=y<y2_r] * [x1_r<=x<x2_r]
             = (diag(s_c) @ U).T @ V   with U[r,y], V[r,x] 0/1 window masks.
  The loss needs only: per-channel global min/max of M, the stride-8 subsample
  of the normalized M (threshold 0.5), and log-reductions of blob_conv.
  Only channels with labels==1 need M at all; invalid channels' loss terms use
  blob_conv alone.

Per-core strategy (8 cores, SPMD):
  - each core computes M for <=VCP valid channels (VCP = ceil(n_valid/8)),
    via on-chip iota-vs-coordinate fp16 masks + PE matmuls into PSUM,
    reducing min/max + subsample straight out of PSUM (no HBM intermediates),
  - blob_conv log terms for invalid channels are round-robined (NIP slots),
  - each core emits one partial scalar; the host sums the 8 partials.
"""

import math
import sys

import numpy as np

for _p in ("/opt/trn_rl_repo",):
    if _p not in sys.path:
        sys.path.append(_p)

EPS = 1e-6
SCORE_THRES = 0.3
BLOB_THRES = 0.5
NCORES = 8

_PROG_CACHE = {}


def _build_program(VCP, NIP, NKT, NB, C):
    import concourse.bacc as bacc
    import concourse.bass as bass
    import concourse.mybir as mybir
    from concourse import tile

    dt = mybir.dt
    f32, f16 = dt.float32, dt.float16
    AF = mybir.ActivationFunctionType
    Op = mybir.AluOpType
    Ax = mybir.AxisListType

    nc = bacc.Bacc("TRN2", target_bir_lowering=False, debug=False,
                   num_devices=NCORES)

    def din(name, shape, dtp=f32):
        return nc.dram_tensor(name, shape, dtp, kind="ExternalInput").ap()

    refine_d = din("refine", [128, NKT * 3 * VCP])
    coords_d = din("coords", [128, 4 * NKT])        # x1 | x2 | y1 | y2 blocks
    xiota_d = din("xiota", [128, 1024], f16)
    yperm_d = din("yperm", [128, 128], f16)
    labels_d = din("labels", [1, C])
    blobp_d = din("blobp", [128, VCP * 128])
    blobpT_d = din("blobpT", [128, VCP * 128])
    blobn_d = din("blobn", [128, NIP * 128])
    blobnT_d = din("blobnT", [128, NIP * 128])
    vp_d = din("vp", [VCP, 1])
    vn_d = din("vn", [NIP, 1])
    out_d = nc.dram_tensor("out", [1, 1], f32, kind="ExternalOutput").ap()

    with tile.TileContext(nc) as tc:
        with (
            tc.tile_pool(name="const", bufs=1) as cp,
            tc.tile_pool(name="work", bufs=2) as wp,
            tc.tile_pool(name="psum", bufs=3, space=bass.MemorySpace.PSUM) as pp,
            tc.tile_pool(name="psums", bufs=1, space=bass.MemorySpace.PSUM) as pps,
        ):
            # ---- load constants / inputs ----
            xiota = cp.tile([128, 1024], f16)
            nc.sync.dma_start(xiota[:], xiota_d)
            yperm = cp.tile([128, 128], f16)
            nc.sync.dma_start(yperm[:], yperm_d)
            coords = cp.tile([128, 4 * NKT], f32)
            nc.sync.dma_start(coords[:], coords_d)
            refS = cp.tile([128, NKT * 3 * VCP], f32)
            nc.sync.dma_start(refS[:], refine_d)
            labels = cp.tile([1, C], f32)
            nc.sync.dma_start(labels[:], labels_d)
            blobp = cp.tile([128, VCP * 128], f32)
            nc.sync.dma_start(blobp[:], blobp_d)
            blobpT = cp.tile([128, VCP * 128], f32)
            nc.sync.dma_start(blobpT[:], blobpT_d)
            blobn = cp.tile([128, NIP * 128], f32)
            nc.sync.dma_start(blobn[:], blobn_d)
            blobnT = cp.tile([128, NIP * 128], f32)
            nc.sync.dma_start(blobnT[:], blobnT_d)
            vp = cp.tile([VCP, 1], f32)
            nc.sync.dma_start(vp[:], vp_d)
            vn = cp.tile([NIP, 1], f32)
            nc.sync.dma_start(vn[:], vn_d)

            ones_r = cp.tile([1, 128], f32)
            nc.vector.memset(ones_r[:], 1.0)
            ones_c = cp.tile([128, 1], f32)
            nc.vector.memset(ones_c[:], 1.0)

            # ---- scores: mean over 3 heads, threshold, to fp16 ----
            ref4 = refS[:].rearrange("p (k h v) -> p k h v", k=NKT, h=3)
            avg = wp.tile([128, NKT * VCP], f32)
            avg3 = avg[:].rearrange("p (k v) -> p k v", k=NKT)
            nc.vector.tensor_add(avg3, ref4[:, :, 0, :], ref4[:, :, 1, :])
            nc.vector.tensor_add(avg3, avg3, ref4[:, :, 2, :])
            nc.vector.tensor_scalar_mul(avg[:], avg[:], 1.0 / 3.0)
            msk = wp.tile([128, NKT * VCP], f32)
            nc.vector.tensor_scalar(msk[:], avg[:], SCORE_THRES, None,
                                    op0=Op.is_ge)
            sc16 = cp.tile([128, NKT * VCP], f16)
            nc.vector.tensor_mul(sc16[:], avg[:], msk[:])
            sc3 = sc16[:].rearrange("p (k v) -> p k v", k=NKT)

            # ---- x window masks, fp16: xwin[r, x] = [x1_r <= x < x2_r] ----
            xwin = cp.tile([128, NKT * 1024], f16)
            xwin3 = xwin[:].rearrange("p (k x) -> p k x", k=NKT)
            for kt in range(NKT):
                x1c = coords[:, kt:kt + 1]
                x2c = coords[:, NKT + kt:NKT + kt + 1]
                ge2 = wp.tile([128, 1024], f16, tag="ge2")
                nc.vector.tensor_scalar(ge2[:], xiota[:], x2c, None,
                                        op0=Op.is_ge)
                nc.vector.scalar_tensor_tensor(
                    xwin3[:, kt, :], xiota[:], x1c, ge2[:],
                    op0=Op.is_ge, op1=Op.subtract)

            colMax = cp.tile([128, VCP * NB], f32)
            colMin = cp.tile([128, VCP * NB], f32)
            Rm = cp.tile([128, VCP * 128], f32)
            Rm16 = cp.tile([16, NB * VCP * 128], f32)
            mxl = cp.tile([128, VCP], f32)
            myl = cp.tile([128, VCP], f32)

            yv_b = yperm[:].rearrange("p (a x) -> p a x", a=1) \
                .broadcast_to([128, NKT, 128])

            # ---- per y-block: build s*U masks, matmul, reduce from PSUM ----
            for t in range(NB):
                y1s = wp.tile([128, NKT], f16, tag="y1s")
                nc.vector.tensor_scalar(y1s[:], coords[:, 2 * NKT:3 * NKT],
                                        float(128 * t), None, op0=Op.subtract)
                y2s = wp.tile([128, NKT], f16, tag="y2s")
                nc.vector.tensor_scalar(y2s[:], coords[:, 3 * NKT:4 * NKT],
                                        float(128 * t), None, op0=Op.subtract)
                y1b = y1s[:].rearrange("p (k a) -> p k a", a=1) \
                    .broadcast_to([128, NKT, 128])
                y2b = y2s[:].rearrange("p (k a) -> p k a", a=1) \
                    .broadcast_to([128, NKT, 128])

                uu = wp.tile([128, NKT * 128], f16, tag="uu")
                uu3 = uu[:].rearrange("p (k y) -> p k y", k=NKT)
                g2 = wp.tile([128, NKT * 128], f16, tag="g2")
                g23 = g2[:].rearrange("p (k y) -> p k y", k=NKT)
                nc.vector.tensor_tensor(g23, yv_b, y2b, op=Op.is_ge)
                nc.vector.tensor_tensor(uu3, yv_b, y1b, op=Op.is_ge)
                nc.vector.tensor_sub(uu[:], uu[:], g2[:])

                for v in range(VCP):
                    su = wp.tile([128, NKT * 128], f16, tag="su")
                    su3 = su[:].rearrange("p (k y) -> p k y", k=NKT)
                    scb = sc3[:, :, v:v + 1].broadcast_to([128, NKT, 128])
                    nc.vector.tensor_tensor(su3, uu3, scb, op=Op.mult)

                    ps = pp.tile([128, 1024], f32, tag="mm")
                    for hh in range(2):
                        for kt in range(NKT):
                            nc.tensor.matmul(
                                ps[:, hh * 512:(hh + 1) * 512],
                                su3[:, kt, :],
                                xwin3[:, kt, hh * 512:(hh + 1) * 512],
                                start=(kt == 0), stop=(kt == NKT - 1))

                    cix = v * NB + t
                    nc.vector.tensor_reduce(colMax[:, cix:cix + 1], ps[:],
                                            axis=Ax.X, op=Op.max)
                    nc.vector.tensor_reduce(colMin[:, cix:cix + 1], ps[:],
                                            axis=Ax.X, op=Op.min)
                    sub_in = ps[0:16, :].rearrange("p (a b) -> p a b", b=8)
                    slot = (t * VCP + v) * 128
                    nc.vector.tensor_copy(Rm16[:, slot:slot + 128],
                                          sub_in[:, :, 0:1])

            # gather the staged subsamples into [y_sub=128, (v, x_sub)] layout
            for t in range(NB):
                nc.sync.dma_start(
                    Rm[16 * t:16 * (t + 1), :],
                    Rm16[:, t * VCP * 128:(t + 1) * VCP * 128]
                    .rearrange("j (v x) -> j v x", v=VCP))

            # ---- normalize subsample, thresholds ----
            for v in range(VCP):
                gmax = wp.tile([1, 1], f32, tag="gmax")
                nc.gpsimd.tensor_reduce(gmax[:],
                                        colMax[:, v * NB:(v + 1) * NB],
                                        axis=Ax.XYZWC, op=Op.max)
                nmin = wp.tile([128, NB], f32, tag="nmin")
                nc.vector.tensor_scalar_mul(nmin[:],
                                            colMin[:, v * NB:(v + 1) * NB],
                                            -1.0)
                gmin_neg = wp.tile([1, 1], f32, tag="gmin")
                nc.gpsimd.tensor_reduce(gmin_neg[:], nmin[:],
                                        axis=Ax.XYZWC, op=Op.max)
                pair = wp.tile([1, 2], f32, tag="pair")
                nc.vector.tensor_add(pair[:, 1:2], gmax[:], gmin_neg[:])
                nc.vector.tensor_scalar(pair[:, 1:2], pair[:, 1:2], EPS, None,
                                        op0=Op.add)
                nc.vector.reciprocal(pair[:, 1:2], pair[:, 1:2])
                nc.vector.tensor_scalar_mul(pair[:, 0:1], gmin_neg[:], -1.0)
                ppair = pps.tile([128, 2], f32, tag="small")
                nc.tensor.matmul(ppair[:], ones_r[:], pair[:],
                                 start=True, stop=True)
                npair = wp.tile([128, 2], f32, tag="npair")
                nc.vector.tensor_copy(npair[:], ppair[:])

                rn16 = wp.tile([128, 128], f16, tag="rn16")
                nc.vector.tensor_scalar(rn16[:], Rm[:, v * 128:(v + 1) * 128],
                                        npair[:, 0:1], npair[:, 1:2],
                                        op0=Op.subtract, op1=Op.mult)
                red = wp.tile([128, 1], f32, tag="red")
                nc.vector.tensor_reduce(red[:], rn16[:], axis=Ax.X, op=Op.max)
                nc.vector.tensor_scalar(myl[:, v:v + 1], red[:], BLOB_THRES,
                                        None, op0=Op.is_ge)
                rnT = wp.tile([128, 128], f16, tag="rnT")
                nc.sync.dma_start_transpose(rnT[:], rn16[:])
                redT = wp.tile([128, 1], f32, tag="redT")
                nc.vector.tensor_reduce(redT[:], rnT[:], axis=Ax.X, op=Op.max)
                nc.vector.tensor_scalar(mxl[:, v:v + 1], redT[:], BLOB_THRES,
                                        None, op0=Op.is_ge)

            # ---- blob side: positive (valid) channels ----
            sbp = wp.tile([128, VCP * 128], f32, tag="sbp")
            nc.vector.tensor_scalar(sbp[:], blobp[:], EPS, 1.0 - EPS,
                                    op0=Op.max, op1=Op.min)
            sbpT = wp.tile([128, VCP * 128], f32, tag="sbpT")
            nc.vector.tensor_scalar(sbpT[:], blobpT[:], EPS, 1.0 - EPS,
                                    op0=Op.max, op1=Op.min)
            myb = wp.tile([128, VCP], f32, tag="myb")
            nc.vector.tensor_reduce(myb[:],
                                    sbp[:].rearrange("p (v w) -> p v w", v=VCP),
                                    axis=Ax.X, op=Op.max)
            mxb = wp.tile([128, VCP], f32, tag="mxb")
            nc.vector.tensor_reduce(mxb[:],
                                    sbpT[:].rearrange("p (v h) -> p v h", v=VCP),
                                    axis=Ax.X, op=Op.max)
            lnx = wp.tile([128, VCP], f32, tag="lnx")
            nc.scalar.activation(lnx[:], mxb[:], AF.Ln)
            lny = wp.tile([128, VCP], f32, tag="lny")
            nc.scalar.activation(lny[:], myb[:], AF.Ln)
            nc.vector.tensor_mul(lnx[:], lnx[:], mxl[:])
            nc.vector.tensor_mul(lny[:], lny[:], myl[:])
            nc.vector.tensor_add(lnx[:], lnx[:], lny[:])
            pv = pps.tile([128, 1], f32, tag="small")
            nc.tensor.matmul(pv[0:VCP, :], lnx[:], ones_c[:], start=True, stop=True)
            spv = wp.tile([VCP, 1], f32, tag="spv")
            nc.vector.tensor_copy(spv[:], pv[0:VCP, :])
            nc.vector.tensor_mul(spv[:], spv[:], vp[:])
            Sp = wp.tile([1, 1], f32, tag="Sp")
            nc.gpsimd.tensor_reduce(Sp[:], spv[:], axis=Ax.XYZWC, op=Op.add)

            # ---- blob side: negative (invalid) channels: ln(1 - x) ----
            sbn = wp.tile([128, NIP * 128], f32, tag="sbn")
            nc.vector.tensor_scalar(sbn[:], blobn[:], EPS, 1.0 - EPS,
                                    op0=Op.max, op1=Op.min)
            sbnT = wp.tile([128, NIP * 128], f32, tag="sbnT")
            nc.vector.tensor_scalar(sbnT[:], blobnT[:], EPS, 1.0 - EPS,
                                    op0=Op.max, op1=Op.min)
            mybn = wp.tile([128, NIP], f32, tag="mybn")
            nc.vector.tensor_reduce(mybn[:],
                                    sbn[:].rearrange("p (v w) -> p v w", v=NIP),
                                    axis=Ax.X, op=Op.max)
            mxbn = wp.tile([128, NIP], f32, tag="mxbn")
            nc.vector.tensor_reduce(mxbn[:],
                                    sbnT[:].rearrange("p (v h) -> p v h", v=NIP),
                                    axis=Ax.X, op=Op.max)
            lnxn = wp.tile([128, NIP], f32, tag="lnxn")
            nc.scalar.activation(lnxn[:], mxbn[:], AF.Ln, bias=1.0, scale=-1.0)
            lnyn = wp.tile([128, NIP], f32, tag="lnyn")
            nc.scalar.activation(lnyn[:], mybn[:], AF.Ln, bias=1.0, scale=-1.0)
            nc.vector.tensor_add(lnxn[:], lnxn[:], lnyn[:])
            nv_ps = pps.tile([128, 1], f32, tag="small")
            nc.tensor.matmul(nv_ps[0:NIP, :], lnxn[:], ones_c[:], start=True,
                             stop=True)
            snv = wp.tile([NIP, 1], f32, tag="snv")
            nc.vector.tensor_copy(snv[:], nv_ps[0:NIP, :])
            nc.vector.tensor_mul(snv[:], snv[:], vn[:])
            Sn = wp.tile([1, 1], f32, tag="Sn")
            nc.gpsimd.tensor_reduce(Sn[:], snv[:], axis=Ax.XYZWC, op=Op.add)

            # ---- divisors from labels, final combine ----
            vmf = wp.tile([1, C], f32, tag="vmf")
            nc.vector.tensor_scalar(vmf[:], labels[:], 1.0, None,
                                    op0=Op.is_equal)
            vc = wp.tile([1, 1], f32, tag="vc")
            nc.vector.tensor_reduce(vc[:], vmf[:], axis=Ax.X, op=Op.add)
            nvc = wp.tile([1, 1], f32, tag="nvc")
            nc.scalar.activation(nvc[:], vc[:], AF.Copy, bias=float(C),
                                 scale=-1.0)
            ivc = wp.tile([1, 1], f32, tag="ivc")
            nc.vector.reciprocal(ivc[:], vc[:])
            invc = wp.tile([1, 1], f32, tag="invc")
            nc.vector.reciprocal(invc[:], nvc[:])
            nc.vector.tensor_mul(Sp[:], Sp[:], ivc[:])
            nc.vector.tensor_mul(Sn[:], Sn[:], invc[:])
            nc.vector.tensor_add(Sp[:], Sp[:], Sn[:])
            tot = wp.tile([1, 1], f32, tag="tot")
            nc.vector.tensor_scalar_mul(tot[:], Sp[:], -1.0 / 128.0)
            nc.sync.dma_start(out_d, tot[:])

    nc.compile()
    return nc


def _get_program(key):
    if key not in _PROG_CACHE:
        _PROG_CACHE[key] = _build_program(*key)
    return _PROG_CACHE[key]


def make_in_maps(mil_result, refine_result, blob_conv, rois, labels, H, W):
    """Host-side sharding: slice/relayout full inputs into 8 per-core maps."""
    refine = np.asarray(refine_result, np.float32)
    blob = np.asarray(blob_conv, np.float32)
    rois = np.asarray(rois, np.float32)
    labels = np.asarray(labels)
    K, R, C1 = refine.shape
    C = labels.shape[1]
    assert int(H) == 1024 and int(W) == 1024
    h, w = blob.shape[-2:]
    assert h == 128 and w == 128

    base = 1 if C1 != C else 0
    valid = labels[0] == 1
    vidx = np.nonzero(valid)[0]
    iidx = np.nonzero(~valid)[0]
    nv, ni = len(vidx), len(iidx)
    VCP = max(1, math.ceil(nv / NCORES))
    NIP = max(1, math.ceil(ni / NCORES))
    RP = math.ceil(R / 128) * 128
    NKT = RP // 128
    NB = 1024 // 128

    b = rois[:, 1:5].astype(np.int32)  # int() truncation, like the reference
    coords = np.zeros((128, 4 * NKT), np.float32)
    for j, arr in enumerate([b[:, 0], b[:, 2], b[:, 1], b[:, 3]]):  # x1 x2 y1 y2
        padv = np.zeros(RP, np.float32)
        padv[:R] = arr
        coords[:, j * NKT:(j + 1) * NKT] = padv.reshape(NKT, 128).T

    xiota = np.ascontiguousarray(
        np.broadcast_to(np.arange(1024, dtype=np.float16), (128, 1024)))
    ysel = np.concatenate([np.arange(0, 128, 8),
                           np.array([j for j in range(128) if j % 8 != 0])])
    yperm = np.ascontiguousarray(
        np.broadcast_to(ysel.astype(np.float16), (128, 128)))
    labels_f = labels.astype(np.float32).reshape(1, C)

    in_maps = []
    for core in range(NCORES):
        refc = np.zeros((128, NKT, 3, VCP), np.float32)
        blobp = np.zeros((128, VCP, 128), np.float32)
        blobpT = np.zeros((128, VCP, 128), np.float32)
        vp = np.zeros((VCP, 1), np.float32)
        for v in range(VCP):
            gi = core + NCORES * v
            if gi < nv:
                ch = int(vidx[gi])
                col = np.zeros((3, RP), np.float32)
                col[:, :R] = refine[:, :, base + ch]
                refc[:, :, :, v] = col.reshape(3, NKT, 128).transpose(2, 1, 0)
                blobp[:, v, :] = blob[ch]
                blobpT[:, v, :] = blob[ch].T
                vp[v, 0] = 1.0
        blobn = np.zeros((128, NIP, 128), np.float32)
        blobnT = np.zeros((128, NIP, 128), np.float32)
        vn = np.zeros((NIP, 1), np.float32)
        for v in range(NIP):
            gi = core + NCORES * v
            if gi < ni:
                ch = int(iidx[gi])
                blobn[:, v, :] = blob[ch]
                blobnT[:, v, :] = blob[ch].T
                vn[v, 0] = 1.0
        in_maps.append({
            "refine": np.ascontiguousarray(refc.reshape(128, -1)),
            "coords": coords,
            "xiota": xiota,
            "yperm": yperm,
            "labels": labels_f,
            "blobp": np.ascontiguousarray(blobp.reshape(128, -1)),
            "blobpT": np.ascontiguousarray(blobpT.reshape(128, -1)),
            "blobn": np.ascontiguousarray(blobn.reshape(128, -1)),
            "blobnT": np.ascontiguousarray(blobnT.reshape(128, -1)),
            "vp": vp,
            "vn": vn,
        })
    key = (VCP, NIP, NKT, NB, C)
    return key, in_maps


def kernel(mil_result, refine_result, blob_conv, rois, labels, H, W,
           _trace=False):
    from concourse.bass_utils import run_bass_kernel_spmd

    key, in_maps = make_in_maps(mil_result, refine_result, blob_conv, rois,
                                labels, H, W)
    nc = _get_program(key)
    res = run_bass_kernel_spmd(nc, in_maps, core_ids=list(range(NCORES)),
                               trace=_trace)
    total = np.float64(0.0)
    for r in res.results:
        total += np.float64(r["out"][0, 0])
    out = np.array(total, dtype=np.float32)
    if _trace:
        kernel.last_results = res
    return out
